# revision 3
# baseline (speedup 1.0000x reference)
"""Trainium2 Bass kernel for ChunkTriangleAttentionStartingNode.

Computation (B=1, N=384, D=128, h=4, c=32):
  Z = LayerNorm(Z_raw) * ln_w + ln_b                     (over d_pair)
  bias[h,q,k]   = (Z @ W_b)[q,k,h]        (triangle bias, row-indexed by q)
  q,k,v         = split(Z @ W_qkv)        per pair-row i, heads h, dim c
  logits[i,h,q,k] = q.k / sqrt(c) + mask_bias[i,k] + bias[h,q,k]
  out = Z_raw + (sigmoid(Z@W_gate + gb) * softmax(logits) @ v) @ W_o + out_bias

Sharding: rows (first pair axis) split across 8 cores, 48 rows each; each
core computes its bias shard, AllGather produces the full [h,N,N] bias.

v1 rework (from the 689us baseline, trace-driven):
  - Phase 1 LN stats via bn_stats/bn_aggr (one DVE pass instead of
    reduce+square+reduce), normalize moved to ACT (per-partition scale+bias
    APs), bias-shard cast moved to ACT.  Kills the SQUARE/SQRT ACT-table
    thrash (16 table loads) and halves phase-1 DVE time.
  - exp(bias^T) precomputed once (exp(l+b) = exp(l)*exp(b)); Eb transposes
    run kc-major so the kc=0 slab is ready first.
  - Pre/post software pipeline with LAG rows: projections/casts of rows
    0..LAG-1 are issued before the Eb section, so they execute during the
    AllGather window instead of idling.
  - out_bias folded into the output-projection PSUM via a rank-1 matmul
    (ones[1,P] x ob[1,N]) accumulation: removes 3 DVE adds per row.
  - v cast and gate tanh on ACT, q/k cast on DVE (engine balance).
  - PSUM: tagA/tagB hold projections, then QK logits per half, then (tagC)
    out_ps; acc holds wa+sum accumulators; 8 banks exactly.
  - QK(kc+1) issued before wa/sum(kc) so the PE never head-of-line blocks
    on the exp/mul of the current chunk.
"""

import os
import sys

os.environ.setdefault("NEURON_RT_RESET_CORES", "1")

for _p in ("/opt/trn_rl_repo",):
    if _p not in sys.path:
        sys.path.append(_p)

import numpy as np
import ml_dtypes

import concourse.bass as bass
import concourse.bacc as bacc
import concourse.tile as tile
from concourse import mybir

F32 = mybir.dt.float32
BF16 = mybir.dt.bfloat16
AF = mybir.ActivationFunctionType
ALU = mybir.AluOpType
AX = mybir.AxisListType

P = 128          # partitions
D = 128          # d_pair
NH = 4           # heads
CH = 32          # head dim
HC = NH * CH     # 128

LAG = int(os.environ.get("K_LAG", "6"))


def build_nc(N=384, n_cores=8):
    C3 = N // P           # chunks along the attention axis
    R = N // n_cores      # rows per core
    lag = min(LAG, R - 1)

    nc = bacc.Bacc(
        "TRN2",
        target_bir_lowering=False,
        debug=False,
        enable_asserts=False,
        num_devices=n_cores,
    )

    Zr = nc.dram_tensor("z_raw", [R, N, D], F32, kind="ExternalInput").ap()
    Zm = nc.dram_tensor("z_mask", [R, N], F32, kind="ExternalInput").ap()
    lnw_d = nc.dram_tensor("ln_w", [D], F32, kind="ExternalInput").ap()
    lnb_d = nc.dram_tensor("ln_b", [D], F32, kind="ExternalInput").ap()
    wb_d = nc.dram_tensor("w_b", [D, NH], F32, kind="ExternalInput").ap()
    wqkv_d = nc.dram_tensor("w_qkv", [D, 3 * HC], F32, kind="ExternalInput").ap()
    wg_d = nc.dram_tensor("w_gate", [D, HC], F32, kind="ExternalInput").ap()
    gb_d = nc.dram_tensor("gating_bias", [HC], F32, kind="ExternalInput").ap()
    wo_d = nc.dram_tensor("w_o", [HC, D], F32, kind="ExternalInput").ap()
    ob_d = nc.dram_tensor("out_bias", [D], F32, kind="ExternalInput").ap()
    OUT = nc.dram_tensor("out", [R, N, D], F32, kind="ExternalOutput").ap()

    id_bf_d = nc.inline_tensor(np.eye(P, dtype=ml_dtypes.bfloat16), "id_bf_c").ap()
    ones_d = nc.inline_tensor(
        np.full((P, CH), 2.0, dtype=ml_dtypes.bfloat16), "ones_c"
    ).ap()

    with tile.TileContext(nc) as tc:
        with (
            tc.tile_pool(name="const", bufs=1) as constp,
            tc.tile_pool(name="res", bufs=1) as resp,
            tc.tile_pool(name="work", bufs=3) as work,
            tc.tile_pool(name="stat", bufs=4) as statp,
            tc.tile_pool(name="wpool", bufs=4) as wpool,
            tc.tile_pool(name="pre", bufs=lag + 2) as prep,
            tc.tile_pool(name="ps", bufs=1, space="PSUM") as psum,
            tc.tile_pool(name="dram", bufs=1, space="DRAM") as dramp,
        ):
            # ---- constants / weights ----
            id_bf = constp.tile([P, P], BF16)
            nc.sync.dma_start(id_bf, id_bf_d)
            ones_bf = constp.tile([P, CH], BF16)
            nc.sync.dma_start(ones_bf, ones_d)

            lnw = constp.tile([D, 1], F32)
            nc.sync.dma_start(lnw, lnw_d[:, None])
            lnb = constp.tile([D, 1], F32)
            nc.sync.dma_start(lnb, lnb_d[:, None])
            gb = constp.tile([HC, 1], F32)
            nc.sync.dma_start(gb, gb_d[:, None])
            ngb = constp.tile([HC, 1], F32)
            nc.scalar.mul(ngb, gb, 0.5)
            eps_c = constp.tile([P, 1], F32)
            nc.gpsimd.memset(eps_c, 1e-5)
            neg1e9_c = constp.tile([P, 1], F32)
            nc.gpsimd.memset(neg1e9_c, -1e9)

            # rank-1 out_bias accumulation operands
            ones1 = constp.tile([1, P], F32)
            nc.gpsimd.memset(ones1, 1.0)
            obr3 = constp.tile([1, C3, P], F32, tag="obr3")
            for c in range(C3):
                nc.sync.dma_start(obr3[:, c, :], ob_d[None, :])

            wtmp = constp.tile([D, 3 * HC], F32, tag="wtmp")
            nc.sync.dma_start(wtmp, wqkv_d)
            wq = constp.tile([D, HC], BF16)
            nc.scalar.activation(wq, wtmp[:, 0:HC], AF.Copy, scale=CH ** -0.5)
            wk = constp.tile([D, HC], BF16)
            nc.scalar.copy(wk, wtmp[:, HC:2 * HC])
            wv = constp.tile([D, HC], BF16)
            nc.scalar.copy(wv, wtmp[:, 2 * HC:3 * HC])

            wgt = constp.tile([D, HC], F32, tag="wgt")
            nc.sync.dma_start(wgt, wg_d)
            wg = constp.tile([D, HC], BF16)
            nc.scalar.copy(wg, wgt)
            wot = constp.tile([HC, D], F32, tag="wot")
            nc.sync.dma_start(wot, wo_d)
            wo = constp.tile([HC, D], BF16)
            nc.scalar.copy(wo, wot)
            wbt = constp.tile([D, NH], F32, tag="wbt")
            nc.sync.dma_start(wbt, wb_d)
            wb = constp.tile([D, NH], BF16)
            nc.scalar.copy(wb, wbt)

            # mask bias columns: mb[kc][k, i] = (Z_mask[i, k] - 1) * 1e9
            mb = []
            for kc in range(C3):
                mk = work.tile([P, R], F32, tag="mk")
                nc.sync.dma_start(
                    mk, Zm[:, kc * P:(kc + 1) * P].rearrange("r p -> p r")
                )
                mbt = resp.tile([P, R], F32, tag=f"mb{kc}", name=f"mb{kc}")
                nc.scalar.activation(mbt, mk, AF.Identity, scale=1e9, bias=neg1e9_c)
                mb.append(mbt)

            # DRAM bounce buffers for the bias AllGather
            b_shard = dramp.tile([R, NH, N], BF16, tag="bshard")
            b_full = dramp.tile(
                [n_cores * R, NH, N], BF16, tag="bfull", addr_space="Shared"
            )

            # ---- phase 1: LayerNorm -> resident Z^T, bias shard ----
            Zt = resp.tile([P, R * C3 * P], BF16, tag="Zt")
            for q in range(R):
                zrow = work.tile([P, C3, P], F32, tag="zrow")
                nc.sync.dma_start(zrow, Zr[q].rearrange("(c p) d -> p c d", p=P))
                st6 = statp.tile([P, C3, 6], F32, tag="st6")
                msv = statp.tile([P, C3, 2], F32, tag="msv")
                for c in range(C3):
                    nc.vector.bn_stats(st6[:, c, :], zrow[:, c, :])
                    nc.vector.bn_aggr(msv[:, c, :], st6[:, c, :])
                std = statp.tile([P, C3], F32, tag="std")
                nc.scalar.activation(std, msv[:, :, 1], AF.Sqrt, bias=eps_c)
                rsig = statp.tile([P, C3], F32, tag="rsig")
                nc.vector.reciprocal(rsig, std)
                nmr = statp.tile([P, C3], F32, tag="nmr")
                nc.vector.scalar_tensor_tensor(
                    nmr, msv[:, :, 0], -1.0, rsig, op0=ALU.mult, op1=ALU.mult
                )
                tp = psum.tile([P, C3, P], BF16, tag="tc", bufs=2, name="tp")
                for c in range(C3):
                    zn = work.tile([P, P], BF16, tag="zn")
                    nc.scalar.activation(
                        zn, zrow[:, c, :], AF.Identity,
                        bias=nmr[:, c:c + 1], scale=rsig[:, c:c + 1],
                    )
                    nc.tensor.transpose(tp[:, c, :], zn, id_bf)
                nc.vector.tensor_scalar(
                    Zt[:, q * C3 * P:(q + 1) * C3 * P].rearrange(
                        "p (c q2) -> p c q2", c=C3
                    ),
                    tp, lnw, lnb, op0=ALU.mult, op1=ALU.add,
                )
                bp = psum.tile([NH, N], F32, tag="tc", bufs=2, name="bp")
                nc.tensor.matmul(bp, wb, Zt[:, q * C3 * P:(q + 1) * C3 * P])
                bsb = work.tile([NH, N], BF16, tag="bsb")
                nc.scalar.copy(bsb, bp)
                nc.sync.dma_start(b_shard[q], bsb)

            nc.gpsimd.collective_compute(
                "AllGather",
                ALU.bypass,
                replica_groups=[list(range(n_cores))],
                ins=[b_shard.opt()],
                outs=[b_full.opt()],
            )
            # bias blocks land via the gpsimd queue so they don't head-of-line
            # block the sync DMA queue while the collective runs
            bt = []
            for qc in range(C3):
                btq = resp.tile([P, NH, N], BF16, tag=f"bt{qc}", name=f"bt{qc}")
                nc.gpsimd.dma_start(btq, b_full[qc * P:(qc + 1) * P])
                bt.append(btq)

            # ---------- pre/post row pipeline pieces ----------
            zrow2s = [None] * R
            qk_sbs = [None] * R
            vsbs = [None] * R
            ths = [None] * R

            def pre(i):
                zrow2 = prep.tile([P, C3, P], F32, tag="zrow2")
                nc.sync.dma_start(zrow2, Zr[i].rearrange("(c p) d -> p c d", p=P))
                zrow2s[i] = zrow2
                zt_row = Zt[:, i * C3 * P:(i + 1) * C3 * P]
                pjA = psum.tile([P, 2, 512], F32, tag="tagA", bufs=1, name="pjA")
                nc.tensor.matmul(pjA[:, 0, 0:N], wq, zt_row)
                nc.tensor.matmul(pjA[:, 1, 0:N], wk, zt_row)
                pjB = psum.tile([P, 2, 512], F32, tag="tagB", bufs=1, name="pjB")
                nc.tensor.matmul(pjB[:, 1, 0:N], wg, zt_row)
                for c in range(C3):
                    nc.tensor.matmul(
                        pjB[:, 0, c * P:(c + 1) * P],
                        zt_row[:, c * P:(c + 1) * P],
                        wv,
                    )
                qk_sb = prep.tile([P, 2, N], BF16, tag="qk_sb")
                nc.vector.tensor_copy(qk_sb, pjA[:, :, 0:N])
                qk_sbs[i] = qk_sb
                vsb = prep.tile([P, C3, P], BF16, tag="vsb")
                nc.scalar.copy(
                    vsb, pjB[:, 0, 0:N].rearrange("p (c q2) -> p c q2", c=C3)
                )
                vsbs[i] = vsb
                th = prep.tile([P, N], BF16, tag="th")
                nc.scalar.activation(th, pjB[:, 1, 0:N], AF.Tanh, scale=0.5, bias=ngb)
                ths[i] = th

            def post(i):
                qt = qk_sbs[i][:, 0, :]
                kt = qk_sbs[i][:, 1, :]
                vsb = vsbs[i]
                wap3 = psum.tile([P, 2, 512], F32, tag="acc", bufs=1, name="wap3")
                wap = wap3[:, 0, 0:N]
                sp = wap3[:, 1, 0:N]

                wms = [None] * C3

                def qk_chunk(kc):
                    w_t4 = wpool.tile([P, NH, N], BF16, tag="wt")
                    wm4 = wpool.tile([P, NH, N], BF16, tag="wm")
                    for half in (0, 1):
                        lg = psum.tile(
                            [P, 2, 512], F32,
                            tag="tagA" if half == 0 else "tagB",
                            bufs=1, name=f"lg{half}",
                        )
                        for hh in range(2):
                            h = half * 2 + hh
                            nc.tensor.matmul(
                                lg[:, hh, 0:N],
                                kt[CH * h:CH * (h + 1), kc * P:(kc + 1) * P],
                                qt[CH * h:CH * (h + 1), :],
                                tile_position=(CH * h, 0),
                            )
                        nc.scalar.activation(
                            w_t4[:, 2 * half:2 * half + 2, :],
                            lg[:, :, 0:N], AF.Exp, bias=mb[kc][:, i:i + 1],
                        )
                        nc.vector.tensor_mul(
                            wm4[:, 2 * half:2 * half + 2, :],
                            w_t4[:, 2 * half:2 * half + 2, :],
                            Eb[kc][:, 2 * half:2 * half + 2, :],
                        )
                    wms[kc] = wm4

                def wa_chunk(kc):
                    wm4 = wms[kc]
                    for h in range(NH):
                        nc.tensor.matmul(
                            wap[CH * h:CH * (h + 1), :],
                            vsb[:, kc, CH * h:CH * (h + 1)],
                            wm4[:, h, :],
                            start=(kc == 0),
                            stop=(kc == C3 - 1),
                            skip_group_check=True,
                            tile_position=(0, CH * h),
                        )
                    for h in range(NH):
                        nc.tensor.matmul(
                            sp[CH * h:CH * (h + 1), :],
                            ones_bf,
                            wm4[:, h, :],
                            start=(kc == 0),
                            stop=(kc == C3 - 1),
                            skip_group_check=True,
                            tile_position=(0, CH * h),
                        )

                # stagger: QK(kc+1) issues before wa/sum(kc) so the PE stream
                # never stalls behind the exp/mul of the current chunk
                qk_chunk(0)
                for kc in range(1, C3):
                    qk_chunk(kc)
                    wa_chunk(kc - 1)
                wa_chunk(C3 - 1)

                rs = work.tile([P, N], F32, tag="rs")
                nc.vector.reciprocal_approx_fast(rs, sp)
                wan = work.tile([P, N], F32, tag="wan")
                nc.vector.tensor_mul(wan, wap, rs)
                gwa = work.tile([P, N], BF16, tag="gwa")
                nc.vector.scalar_tensor_tensor(
                    gwa, ths[i], 1.0, wan, op0=ALU.add, op1=ALU.mult
                )
                out_ps = psum.tile([P, C3, P], F32, tag="tc", bufs=2, name="out_ps")
                nc.tensor.matmul(
                    out_ps.rearrange("p c d -> p (c d)"),
                    ones1, obr3.rearrange("o c d -> o (c d)"),
                    start=True, stop=False, skip_group_check=True,
                )
                for c in range(C3):
                    nc.tensor.matmul(
                        out_ps[:, c, :], gwa[:, c * P:(c + 1) * P], wo,
                        start=False, stop=True, skip_group_check=True,
                    )
                fin = work.tile([P, C3, P], F32, tag="fin")
                nc.vector.tensor_add(fin, out_ps, zrow2s[i])
                nc.sync.dma_start(OUT[i].rearrange("(c p) d -> p c d", p=P), fin)
                zrow2s[i] = qk_sbs[i] = vsbs[i] = ths[i] = None

            # issue the first LAG rows' projections before the Eb section so
            # they run during the AllGather window
            for i in range(lag):
                pre(i)

            # exp of transposed bias, resident per k-chunk: Eb[kc][k, h, q];
            # kc-major so Eb[0] (needed by the first post) completes first
            Eb = [
                resp.tile([P, NH, N], BF16, tag=f"eb{kc}", name=f"eb{kc}")
                for kc in range(C3)
            ]
            for kc in range(C3):
                for qc in range(C3):
                    for h in range(NH):
                        tp2 = psum.tile([P, P], BF16, tag="tc", bufs=2, name="tp2")
                        nc.tensor.transpose(
                            tp2, bt[qc][:, h, kc * P:(kc + 1) * P], id_bf
                        )
                        nc.scalar.activation(
                            Eb[kc][:, h, qc * P:(qc + 1) * P], tp2, AF.Exp
                        )

            # ---- phase 2: per-row attention, software-pipelined ----
            for i in range(R):
                post(i)
                if i + lag < R:
                    pre(i + lag)

    nc.compile()
    return nc


_CACHE = {}


def get_nc(N=384, n_cores=8):
    key = (N, n_cores)
    if key not in _CACHE:
        _CACHE[key] = build_nc(N, n_cores)
    return _CACHE[key]


def make_in_maps(inputs, N=384, n_cores=8):
    R = N // n_cores
    Z = np.ascontiguousarray(np.asarray(inputs["Z_raw"], dtype=np.float32))
    M = np.ascontiguousarray(np.asarray(inputs["Z_mask"], dtype=np.float32))
    shared = {
        "ln_w": np.ascontiguousarray(np.asarray(inputs["ln_w"], np.float32)),
        "ln_b": np.ascontiguousarray(np.asarray(inputs["ln_b"], np.float32)),
        "w_b": np.ascontiguousarray(np.asarray(inputs["W_b"], np.float32)),
        "w_qkv": np.ascontiguousarray(np.asarray(inputs["W_qkv"], np.float32)),
        "w_gate": np.ascontiguousarray(np.asarray(inputs["W_gate"], np.float32)),
        "gating_bias": np.ascontiguousarray(
            np.asarray(inputs["gating_bias"], np.float32)
        ),
        "w_o": np.ascontiguousarray(np.asarray(inputs["W_o"], np.float32)),
        "out_bias": np.ascontiguousarray(np.asarray(inputs["out_bias"], np.float32)),
    }
    in_maps = []
    for c in range(n_cores):
        m = dict(shared)
        m["z_raw"] = np.ascontiguousarray(Z[0, c * R:(c + 1) * R])
        m["z_mask"] = np.ascontiguousarray(M[0, c * R:(c + 1) * R])
        in_maps.append(m)
    return in_maps


def kernel(**inputs):
    from concourse.bass_utils import run_bass_kernel_spmd

    N, n_cores = 384, 8
    nc = get_nc(N, n_cores)
    in_maps = make_in_maps(inputs, N, n_cores)
    res = run_bass_kernel_spmd(nc, in_maps, list(range(n_cores)))
    out = np.concatenate([res.results[c]["out"] for c in range(n_cores)], axis=0)
    return out.reshape(1, N, N, D).astype(np.float32)


# revision 5
# speedup vs baseline: 1.2977x; 1.2977x over previous
"""Trainium2 Bass kernel for ChunkTriangleAttentionStartingNode.

Computation (B=1, N=384, D=128, h=4, c=32):
  Z = LayerNorm(Z_raw) * ln_w + ln_b                     (over d_pair)
  bias[h,q,k]   = (Z @ W_b)[q,k,h]        (triangle bias, row-indexed by q)
  q,k,v         = split(Z @ W_qkv)        per pair-row i, heads h, dim c
  logits[i,h,q,k] = q.k / sqrt(c) + mask_bias[i,k] + bias[h,q,k]
  out = Z_raw + (sigmoid(Z@W_gate + gb) * softmax(logits) @ v) @ W_o + out_bias

Sharding: rows (first pair axis) split across 8 cores, 48 rows each; each
core computes its bias shard, AllGather produces the full [h,N,N] bias.

v1 rework (from the 689us baseline, trace-driven):
  - Phase 1 LN stats via bn_stats/bn_aggr (one DVE pass instead of
    reduce+square+reduce), normalize moved to ACT (per-partition scale+bias
    APs), bias-shard cast moved to ACT.  Kills the SQUARE/SQRT ACT-table
    thrash (16 table loads) and halves phase-1 DVE time.
  - exp(bias^T) precomputed once (exp(l+b) = exp(l)*exp(b)); Eb transposes
    run kc-major so the kc=0 slab is ready first.
  - Pre/post software pipeline with LAG rows: projections/casts of rows
    0..LAG-1 are issued before the Eb section, so they execute during the
    AllGather window instead of idling.
  - out_bias folded into the output-projection PSUM via a rank-1 matmul
    (ones[1,P] x ob[1,N]) accumulation: removes 3 DVE adds per row.
  - v cast and gate tanh on ACT, q/k cast on DVE (engine balance).
  - PSUM: tagA/tagB hold projections, then QK logits per half, then (tagC)
    out_ps; acc holds wa+sum accumulators; 8 banks exactly.
  - QK(kc+1) issued before wa/sum(kc) so the PE never head-of-line blocks
    on the exp/mul of the current chunk.
"""

import os
import sys

os.environ.setdefault("NEURON_RT_RESET_CORES", "1")

for _p in ("/opt/trn_rl_repo",):
    if _p not in sys.path:
        sys.path.append(_p)

import numpy as np
import ml_dtypes

import concourse.bass as bass
import concourse.bacc as bacc
import concourse.tile as tile
from concourse import mybir

F32 = mybir.dt.float32
BF16 = mybir.dt.bfloat16
AF = mybir.ActivationFunctionType
ALU = mybir.AluOpType
AX = mybir.AxisListType

P = 128          # partitions
D = 128          # d_pair
NH = 4           # heads
CH = 32          # head dim
HC = NH * CH     # 128

LAG = int(os.environ.get("K_LAG", "6"))


def build_nc(N=384, n_cores=8):
    C3 = N // P           # chunks along the attention axis
    R = N // n_cores      # rows per core
    lag = min(LAG, R - 1)

    nc = bacc.Bacc(
        "TRN2",
        target_bir_lowering=False,
        debug=False,
        enable_asserts=False,
        num_devices=n_cores,
    )

    Zr = nc.dram_tensor("z_raw", [R, N, D], F32, kind="ExternalInput").ap()
    Zm = nc.dram_tensor("z_mask", [R, N], F32, kind="ExternalInput").ap()
    lnw_d = nc.dram_tensor("ln_w", [D], F32, kind="ExternalInput").ap()
    lnb_d = nc.dram_tensor("ln_b", [D], F32, kind="ExternalInput").ap()
    wb_d = nc.dram_tensor("w_b", [D, NH], F32, kind="ExternalInput").ap()
    wqkv_d = nc.dram_tensor("w_qkv", [D, 3 * HC], F32, kind="ExternalInput").ap()
    wg_d = nc.dram_tensor("w_gate", [D, HC], F32, kind="ExternalInput").ap()
    gb_d = nc.dram_tensor("gating_bias", [HC], F32, kind="ExternalInput").ap()
    wo_d = nc.dram_tensor("w_o", [HC, D], F32, kind="ExternalInput").ap()
    ob_d = nc.dram_tensor("out_bias", [D], F32, kind="ExternalInput").ap()
    OUT = nc.dram_tensor("out", [R, N, D], F32, kind="ExternalOutput").ap()

    id_bf_d = nc.inline_tensor(np.eye(P, dtype=ml_dtypes.bfloat16), "id_bf_c").ap()
    ones_d = nc.inline_tensor(
        np.full((P, CH), 2.0, dtype=ml_dtypes.bfloat16), "ones_c"
    ).ap()

    with tile.TileContext(nc) as tc:
        with (
            tc.tile_pool(name="const", bufs=1) as constp,
            tc.tile_pool(name="res", bufs=1) as resp,
            tc.tile_pool(name="work", bufs=3) as work,
            tc.tile_pool(name="stat", bufs=4) as statp,
            tc.tile_pool(name="wpool", bufs=4) as wpool,
            tc.tile_pool(name="pre", bufs=lag + 2) as prep,
            tc.tile_pool(name="ps", bufs=1, space="PSUM") as psum,
            tc.tile_pool(name="dram", bufs=1, space="DRAM") as dramp,
        ):
            # ---- constants / weights ----
            id_bf = constp.tile([P, P], BF16)
            nc.sync.dma_start(id_bf, id_bf_d)
            ones_bf = constp.tile([P, CH], BF16)
            nc.sync.dma_start(ones_bf, ones_d)

            lnw = constp.tile([D, 1], F32)
            nc.sync.dma_start(lnw, lnw_d[:, None])
            lnb = constp.tile([D, 1], F32)
            nc.sync.dma_start(lnb, lnb_d[:, None])
            gb = constp.tile([HC, 1], F32)
            nc.sync.dma_start(gb, gb_d[:, None])
            ngb = constp.tile([HC, 1], F32)
            nc.scalar.mul(ngb, gb, 0.5)
            eps_c = constp.tile([P, 1], F32)
            nc.gpsimd.memset(eps_c, 1e-5)
            neg1e9_c = constp.tile([P, 1], F32)
            nc.gpsimd.memset(neg1e9_c, -1e9)

            # rank-1 out_bias accumulation operands
            ones1 = constp.tile([1, P], F32)
            nc.gpsimd.memset(ones1, 1.0)
            obr3 = constp.tile([1, C3, P], F32, tag="obr3")
            for c in range(C3):
                nc.sync.dma_start(obr3[:, c, :], ob_d[None, :])

            wtmp = constp.tile([D, 3 * HC], F32, tag="wtmp")
            nc.sync.dma_start(wtmp, wqkv_d)
            wq = constp.tile([D, HC], BF16)
            nc.scalar.activation(wq, wtmp[:, 0:HC], AF.Copy, scale=CH ** -0.5)
            wk = constp.tile([D, HC], BF16)
            nc.scalar.copy(wk, wtmp[:, HC:2 * HC])
            wv = constp.tile([D, HC], BF16)
            nc.scalar.copy(wv, wtmp[:, 2 * HC:3 * HC])

            wgt = constp.tile([D, HC], F32, tag="wgt")
            nc.sync.dma_start(wgt, wg_d)
            wg = constp.tile([D, HC], BF16)
            nc.scalar.copy(wg, wgt)
            wot = constp.tile([HC, D], F32, tag="wot")
            nc.sync.dma_start(wot, wo_d)
            wo = constp.tile([HC, D], BF16)
            nc.scalar.copy(wo, wot)
            wbt = constp.tile([D, NH], F32, tag="wbt")
            nc.sync.dma_start(wbt, wb_d)
            wb = constp.tile([D, NH], BF16)
            nc.scalar.copy(wb, wbt)

            # mask bias columns: mb[kc][k, i] = (Z_mask[i, k] - 1) * 1e9
            mb = []
            for kc in range(C3):
                mk = work.tile([P, R], F32, tag="mk")
                nc.sync.dma_start(
                    mk, Zm[:, kc * P:(kc + 1) * P].rearrange("r p -> p r")
                )
                mbt = resp.tile([P, R], F32, tag=f"mb{kc}", name=f"mb{kc}")
                nc.scalar.activation(mbt, mk, AF.Identity, scale=1e9, bias=neg1e9_c)
                mb.append(mbt)

            # DRAM bounce buffers for the bias AllGather
            b_shard = dramp.tile([R, NH, N], BF16, tag="bshard")
            b_full = dramp.tile(
                [n_cores * R, NH, N], BF16, tag="bfull", addr_space="Shared"
            )

            # ---- phase 1a: LN stats for ALL rows (DVE), then one batched
            # sqrt -> rsig/nmr.  A single Sqrt instruction means the
            # scheduler cannot interleave it with exp/tanh-set activations,
            # killing the ACT table-set thrash seen with per-row sqrt.
            msv_all = resp.tile([P, R, C3, 2], F32, tag="msv_all")
            for q in range(R):
                zrowA = work.tile([P, C3, P], F32, tag="zrowA")
                nc.sync.dma_start(zrowA, Zr[q].rearrange("(c p) d -> p c d", p=P))
                st6 = statp.tile([P, C3, 6], F32, tag="st6")
                for c in range(C3):
                    nc.vector.bn_stats(st6[:, c, :], zrowA[:, c, :])
                    nc.vector.bn_aggr(msv_all[:, q, c, :], st6[:, c, :])
            std_all = resp.tile([P, R, C3], F32, tag="std_all")
            nc.scalar.activation(
                std_all.rearrange("p r c -> p (r c)"),
                msv_all[:, :, :, 1].rearrange("p r c -> p (r c)"),
                AF.Sqrt, bias=eps_c,
            )
            rsig_all = resp.tile([P, R, C3], F32, tag="rsig_all")
            nc.vector.reciprocal(
                rsig_all.rearrange("p r c -> p (r c)"),
                std_all.rearrange("p r c -> p (r c)"),
            )
            nmr_all = resp.tile([P, R, C3], F32, tag="nmr_all")
            nc.vector.scalar_tensor_tensor(
                nmr_all.rearrange("p r c -> p (r c)"),
                msv_all[:, :, :, 0].rearrange("p r c -> p (r c)"),
                -1.0,
                rsig_all.rearrange("p r c -> p (r c)"),
                op0=ALU.mult, op1=ALU.mult,
            )

            # ---- phase 1b: normalize -> resident Z^T, bias shard ----
            Zt = resp.tile([P, R * C3 * P], BF16, tag="Zt")
            for q in range(R):
                zrow = work.tile([P, C3, P], F32, tag="zrow")
                nc.sync.dma_start(zrow, Zr[q].rearrange("(c p) d -> p c d", p=P))
                tp = psum.tile([P, C3, P], BF16, tag="tc", bufs=2, name="tp")
                for c in range(C3):
                    zn = work.tile([P, P], BF16, tag="zn")
                    nc.scalar.activation(
                        zn, zrow[:, c, :], AF.Identity,
                        bias=nmr_all[:, q, c:c + 1],
                        scale=rsig_all[:, q, c:c + 1],
                    )
                    nc.tensor.transpose(tp[:, c, :], zn, id_bf)
                nc.vector.tensor_scalar(
                    Zt[:, q * C3 * P:(q + 1) * C3 * P].rearrange(
                        "p (c q2) -> p c q2", c=C3
                    ),
                    tp, lnw, lnb, op0=ALU.mult, op1=ALU.add,
                )
                bp = psum.tile([NH, N], F32, tag="tc", bufs=2, name="bp")
                nc.tensor.matmul(bp, wb, Zt[:, q * C3 * P:(q + 1) * C3 * P])
                bsb = work.tile([NH, N], BF16, tag="bsb")
                nc.vector.tensor_copy(bsb, bp)
                nc.sync.dma_start(b_shard[q], bsb)

            nc.gpsimd.collective_compute(
                "AllGather",
                ALU.bypass,
                replica_groups=[list(range(n_cores))],
                ins=[b_shard.opt()],
                outs=[b_full.opt()],
            )
            bt = []
            for qc in range(C3):
                btq = resp.tile([P, NH, N], BF16, tag=f"bt{qc}", name=f"bt{qc}")
                nc.sync.dma_start(btq, b_full[qc * P:(qc + 1) * P])
                bt.append(btq)

            # ---------- pre/post row pipeline pieces ----------
            zrow2s = [None] * R
            qk_sbs = [None] * R
            vsbs = [None] * R
            ths = [None] * R

            def pre(i):
                zrow2 = prep.tile([P, C3, P], F32, tag="zrow2")
                nc.sync.dma_start(zrow2, Zr[i].rearrange("(c p) d -> p c d", p=P))
                zrow2s[i] = zrow2
                zt_row = Zt[:, i * C3 * P:(i + 1) * C3 * P]
                pjA = psum.tile([P, 2, 512], F32, tag="tagA", bufs=1, name="pjA")
                nc.tensor.matmul(pjA[:, 0, 0:N], wq, zt_row)
                nc.tensor.matmul(pjA[:, 1, 0:N], wk, zt_row)
                pjB = psum.tile([P, 2, 512], F32, tag="tagB", bufs=1, name="pjB")
                nc.tensor.matmul(pjB[:, 1, 0:N], wg, zt_row)
                for c in range(C3):
                    nc.tensor.matmul(
                        pjB[:, 0, c * P:(c + 1) * P],
                        zt_row[:, c * P:(c + 1) * P],
                        wv,
                    )
                qk_sb = prep.tile([P, 2, N], BF16, tag="qk_sb")
                nc.vector.tensor_copy(qk_sb, pjA[:, :, 0:N])
                qk_sbs[i] = qk_sb
                vsb = prep.tile([P, C3, P], BF16, tag="vsb")
                nc.scalar.copy(
                    vsb, pjB[:, 0, 0:N].rearrange("p (c q2) -> p c q2", c=C3)
                )
                vsbs[i] = vsb
                th = prep.tile([P, N], BF16, tag="th")
                nc.scalar.activation(th, pjB[:, 1, 0:N], AF.Tanh, scale=0.5, bias=ngb)
                ths[i] = th

            def post(i):
                qt = qk_sbs[i][:, 0, :]
                kt = qk_sbs[i][:, 1, :]
                vsb = vsbs[i]
                wap3 = psum.tile([P, 2, 512], F32, tag="acc", bufs=1, name="wap3")
                wap = wap3[:, 0, 0:N]
                sp = wap3[:, 1, 0:N]

                wms = [None] * C3

                def qk_chunk(kc):
                    w_t4 = wpool.tile([P, NH, N], BF16, tag="wt")
                    wm4 = wpool.tile([P, NH, N], BF16, tag="wm")
                    for half in (0, 1):
                        lg = psum.tile(
                            [P, 2, 512], F32,
                            tag="tagA" if half == 0 else "tagB",
                            bufs=1, name=f"lg{half}",
                        )
                        for hh in range(2):
                            h = half * 2 + hh
                            nc.tensor.matmul(
                                lg[:, hh, 0:N],
                                kt[CH * h:CH * (h + 1), kc * P:(kc + 1) * P],
                                qt[CH * h:CH * (h + 1), :],
                                tile_position=(CH * h, 0),
                            )
                        nc.scalar.activation(
                            w_t4[:, 2 * half:2 * half + 2, :],
                            lg[:, :, 0:N], AF.Exp, bias=mb[kc][:, i:i + 1],
                        )
                        nc.vector.tensor_mul(
                            wm4[:, 2 * half:2 * half + 2, :],
                            w_t4[:, 2 * half:2 * half + 2, :],
                            Eb[kc][:, 2 * half:2 * half + 2, :],
                        )
                    wms[kc] = wm4

                def wa_chunk(kc):
                    wm4 = wms[kc]
                    for h in range(NH):
                        nc.tensor.matmul(
                            wap[CH * h:CH * (h + 1), :],
                            vsb[:, kc, CH * h:CH * (h + 1)],
                            wm4[:, h, :],
                            start=(kc == 0),
                            stop=(kc == C3 - 1),
                            skip_group_check=True,
                            tile_position=(0, CH * h),
                        )
                    for h in range(NH):
                        nc.tensor.matmul(
                            sp[CH * h:CH * (h + 1), :],
                            ones_bf,
                            wm4[:, h, :],
                            start=(kc == 0),
                            stop=(kc == C3 - 1),
                            skip_group_check=True,
                            tile_position=(0, CH * h),
                        )

                # stagger: QK(kc+1) issues before wa/sum(kc) so the PE stream
                # never stalls behind the exp/mul of the current chunk
                qk_chunk(0)
                for kc in range(1, C3):
                    qk_chunk(kc)
                    wa_chunk(kc - 1)
                wa_chunk(C3 - 1)

                rs = work.tile([P, N], F32, tag="rs")
                nc.vector.reciprocal_approx_fast(rs, sp)
                wan = work.tile([P, N], F32, tag="wan")
                nc.vector.tensor_mul(wan, wap, rs)
                gwa = work.tile([P, N], BF16, tag="gwa")
                nc.vector.scalar_tensor_tensor(
                    gwa, ths[i], 1.0, wan, op0=ALU.add, op1=ALU.mult
                )
                out_ps = psum.tile([P, C3, P], F32, tag="tc", bufs=2, name="out_ps")
                nc.tensor.matmul(
                    out_ps.rearrange("p c d -> p (c d)"),
                    ones1, obr3.rearrange("o c d -> o (c d)"),
                    start=True, stop=False, skip_group_check=True,
                )
                for c in range(C3):
                    nc.tensor.matmul(
                        out_ps[:, c, :], gwa[:, c * P:(c + 1) * P], wo,
                        start=False, stop=True, skip_group_check=True,
                    )
                fin = work.tile([P, C3, P], F32, tag="fin")
                nc.vector.tensor_add(fin, out_ps, zrow2s[i])
                nc.sync.dma_start(OUT[i].rearrange("(c p) d -> p c d", p=P), fin)
                zrow2s[i] = qk_sbs[i] = vsbs[i] = ths[i] = None

            # issue the first LAG rows' projections before the Eb section so
            # they run during the AllGather window
            for i in range(lag):
                pre(i)

            # exp of transposed bias, resident per k-chunk: Eb[kc][k, h, q];
            # kc-major so Eb[0] (needed by the first post) completes first
            Eb = [
                resp.tile([P, NH, N], BF16, tag=f"eb{kc}", name=f"eb{kc}")
                for kc in range(C3)
            ]
            for kc in range(C3):
                for qc in range(C3):
                    for h in range(NH):
                        tp2 = psum.tile([P, P], BF16, tag="tc", bufs=2, name="tp2")
                        nc.tensor.transpose(
                            tp2, bt[qc][:, h, kc * P:(kc + 1) * P], id_bf
                        )
                        nc.scalar.activation(
                            Eb[kc][:, h, qc * P:(qc + 1) * P], tp2, AF.Exp
                        )

            # ---- phase 2: per-row attention, software-pipelined ----
            for i in range(R):
                post(i)
                if i + lag < R:
                    pre(i + lag)

    nc.compile()
    return nc


_CACHE = {}


def get_nc(N=384, n_cores=8):
    key = (N, n_cores)
    if key not in _CACHE:
        _CACHE[key] = build_nc(N, n_cores)
    return _CACHE[key]


def make_in_maps(inputs, N=384, n_cores=8):
    R = N // n_cores
    Z = np.ascontiguousarray(np.asarray(inputs["Z_raw"], dtype=np.float32))
    M = np.ascontiguousarray(np.asarray(inputs["Z_mask"], dtype=np.float32))
    shared = {
        "ln_w": np.ascontiguousarray(np.asarray(inputs["ln_w"], np.float32)),
        "ln_b": np.ascontiguousarray(np.asarray(inputs["ln_b"], np.float32)),
        "w_b": np.ascontiguousarray(np.asarray(inputs["W_b"], np.float32)),
        "w_qkv": np.ascontiguousarray(np.asarray(inputs["W_qkv"], np.float32)),
        "w_gate": np.ascontiguousarray(np.asarray(inputs["W_gate"], np.float32)),
        "gating_bias": np.ascontiguousarray(
            np.asarray(inputs["gating_bias"], np.float32)
        ),
        "w_o": np.ascontiguousarray(np.asarray(inputs["W_o"], np.float32)),
        "out_bias": np.ascontiguousarray(np.asarray(inputs["out_bias"], np.float32)),
    }
    in_maps = []
    for c in range(n_cores):
        m = dict(shared)
        m["z_raw"] = np.ascontiguousarray(Z[0, c * R:(c + 1) * R])
        m["z_mask"] = np.ascontiguousarray(M[0, c * R:(c + 1) * R])
        in_maps.append(m)
    return in_maps


def kernel(**inputs):
    from concourse.bass_utils import run_bass_kernel_spmd

    N, n_cores = 384, 8
    nc = get_nc(N, n_cores)
    in_maps = make_in_maps(inputs, N, n_cores)
    res = run_bass_kernel_spmd(nc, in_maps, list(range(n_cores)))
    out = np.concatenate([res.results[c]["out"] for c in range(n_cores)], axis=0)
    return out.reshape(1, N, N, D).astype(np.float32)
